# revision 39
# baseline (speedup 1.0000x reference)
"""Trainium2 Bass kernel: GQA attention with KV cache (decode, Sq=4).

Problem shapes (hardcoded):
  Q [4, 4, 32, 128] f32, K [4, 8192, 8, 128] f32, V [4, 8192, 8, 128] f32,
  cache_seqlens [4] i32 in [4096, 8192].  Output [4, 4, 32, 128] f32.

Sharding: tensor-parallel over the 8 KV heads — core c owns KV head c and
its 4 grouped query heads, for all 4 batches.  Every core therefore does
identical work regardless of cache_seqlens skew.

Per (batch) unit, per 16-block chunk of the KV cache (block = 128 positions):
  scoresT[s,q] = (K_blk^T as lhsT stationary) x (Q^T moving [128,16])
  p = exp(scoresT)            (no max-subtraction needed: scores ~ N(0,1))
  denom partials: ones[128,1] lhsT x p chunk -> [1, 256] PSUM accumulation
  outT[dv,q] += (V_blk [128,128] as lhsT stationary) x (p_blk moving [128,16])
With V stationary the PV matmul streams only 16 columns per block (like the
scores matmul), so the PE runs at ~34ns/block for both phases instead of
128-column streaming.  The kernel returns the transposed numerator and the
denominator partials; the final division + transpose happen on the host.

All K/V transfers are issued up front on the sync HWDGE ring into fully-
resident SBUF tiles (~115KB/partition total), so the stream runs flat at
the ~420 GB/s per-core HBM ceiling with no buffer-recycling stalls.
Masked tail (last <=2 blocks) is zeroed on p with a host-built 0/1 mask.
Blocks past ceil(cache_seqlens/128)*128 are skipped entirely (sparse win).
"""

import functools

import numpy as np
import ml_dtypes

import concourse.bacc as bacc
import concourse.mybir as mybir
import concourse.tile as tile
from concourse import bass_utils

B, SQ, H, HKV, D, DV, SMAX = 4, 4, 32, 8, 128, 128, 8192
G = H // HKV  # 4 query heads per KV head
QR = SQ * G  # 16 query rows per (batch, kv-head) unit
BLK = 128  # kv positions per matmul block
CH = 16  # blocks per chunk (PSUM-score/exp granularity)
DEN_J = 16  # denominator j-slot count (j mod DEN_J); region [1, DEN_J*QR] in PSUM
NCORES = 8

MM_DT = mybir.dt.bfloat16
MM_NP = np.dtype(ml_dtypes.bfloat16)
F32 = mybir.dt.float32


def _chunks(nblk):
    return [(j0, min(j0 + CH, nblk)) for j0 in range(0, nblk, CH)]


def _lean_drain_and_barrier(self, tick_clock, wait_clock):
    """Cheaper TileContext exit: drain + one barrier + sem/DMA reset, without
    the trailing all-engine barrier.  Nothing follows the TileContext in this
    program, and nrt waits for every engine to halt before re-execution, so
    the semaphore clears still happen-before any subsequent run."""
    from concourse.vector_clock import ScopedClock

    drain_inst = self.nc.sync.drain()
    wait_clock.add_sem_waits(
        drain_inst.ins, ScopedClock({None: tick_clock.global_clock})
    )
    self.nc.all_engine_barrier()
    popped = self.nc._tile_sem_poison_stack.pop()
    assert popped is self._sem_poison
    self.nc.clear_and_free_semaphores(list(self.sems.allocated().values()))


@functools.lru_cache(maxsize=4)
def _build(nblks: tuple[int, ...]):
    """Build + compile the per-core SPMD program for given per-batch block counts."""
    nc = bacc.Bacc("TRN2", target_bir_lowering=False, debug=False)

    qt = nc.dram_tensor("qt", [D, B * QR], MM_DT, kind="ExternalInput")
    kt = [
        nc.dram_tensor(f"kt{b}", [D, n * BLK], MM_DT, kind="ExternalInput")
        for b, n in enumerate(nblks)
    ]
    # V arrives host-swizzled to the SBUF image: [sl, kb*DV] with
    # v[sl, kb*DV + dv] = V[128*kb + sl, dv] — flat contiguous runs/partition.
    v = [
        nc.dram_tensor(f"v{b}", [BLK, n * DV], MM_DT, kind="ExternalInput")
        for b, n in enumerate(nblks)
    ]
    mask = nc.dram_tensor("mask", [BLK, B * 2 * QR], MM_DT, kind="ExternalInput")
    ones = nc.dram_tensor("ones", [BLK, 1], MM_DT, kind="ExternalInput")
    outT = nc.dram_tensor("outT", [B, DV, QR], F32, kind="ExternalOutput")
    den = nc.dram_tensor("den", [B, 1, DEN_J * QR], F32, kind="ExternalOutput")

    tile.TileContext._drain_and_barrier = _lean_drain_and_barrier
    with tile.TileContext(nc) as tc:
        with (
            tc.tile_pool(name="const", bufs=1) as cpool,
            tc.tile_pool(name="ktp", bufs=1) as kpool,
            tc.tile_pool(name="vp", bufs=1) as vpool,
            tc.tile_pool(name="pp", bufs=1) as ppool,
            tc.tile_pool(name="small", bufs=1) as spool,
            tc.tile_pool(name="psT", bufs=4, space="PSUM") as psTpool,
            tc.tile_pool(name="psO", bufs=1, space="PSUM") as psOpool,
        ):
            # qt rides first on the sync ring (needed by the first scores);
            # mask/ones go via gpsimd SWDGE so they never delay the K/V
            # stream issues.
            qt_t = cpool.tile([D, B * QR], MM_DT, tag="qt")
            nc.sync.dma_start(qt_t[:], qt[:])
            mask_t = cpool.tile([BLK, B * 2 * QR], MM_DT, tag="mask")
            nc.gpsimd.dma_start(mask_t[:], mask[:])
            ones_t = cpool.tile([BLK, 1], MM_DT, tag="ones")
            nc.gpsimd.dma_start(ones_t[:], ones[:])

            kt_sb, v_sb, p_u, ps = [], [], [], []
            for b in range(B):
                n = nblks[b]
                kt_sb.append(
                    kpool.tile([D, n * BLK], MM_DT, name=f"ktsb{b}")
                )
                v_sb.append(vpool.tile([BLK, n * DV], MM_DT, name=f"vsb{b}"))
                p_u.append(ppool.tile([BLK, n * QR], MM_DT, name=f"pu{b}"))
                # [:, :QR] holds the outT accumulator; [0:1, QR:QR+DEN_J*QR]
                # holds the denominator partial sums (per j-slot x q).
                ps.append(
                    psOpool.tile([BLK, QR + DEN_J * QR], F32, name=f"ps{b}")
                )

            # All K/V transfers up front on the SYNC ring only, interleaved
            # K-then-V per wave.  Sync does nothing else, so the per-engine
            # DMA-semaphore pacing (first ~8 issues fire back-to-back, later
            # ones block the engine until an older transfer completes) is
            # harmless there.  Putting transfers on the scalar ring instead
            # stalls the ACTIVATEs queued behind the blocked issues —
            # measured as a 7-9us regression, twice.  ~1MB transfers,
            # round-robined across batches per wave so the four per-batch
            # tails land staggered at the stream end and their
            # scores->exp->PV chains overlap.
            DCH = 2 * CH  # blocks per DMA transfer (1MB)
            ntr = max((nblks[b] + DCH - 1) // DCH for b in range(B))

            # b0/b1 go as one monolithic K/V transfer each — their chunk
            # chains hide under the stream, and halving the early issue
            # count (each DMA_DIRECT2D costs 0.6-2us of sync-engine time)
            # gets the ring to full depth sooner, flattening the ramp dip.
            # b2/b3 keep two waves so the terminal chain gates on a small
            # final piece.
            sp3 = ((nblks[3] - 1) // CH) * CH  # b3's last chunk start
            plan = [
                (0, 0, nblks[0]),
                (1, 0, nblks[1]),
                (2, 0, min(DCH, nblks[2])),
                (3, 0, min(DCH, nblks[3])),
                (2, min(DCH, nblks[2]), nblks[2]),
                # b3's tail in two pieces: the terminal piece is just its
                # last compute chunk, so the final PV chain gates on a tiny
                # transfer (~4KB/queue) instead of ~36KB/queue
                (3, min(DCH, nblks[3]), max(sp3, min(DCH, nblks[3]))),
                (3, max(sp3, min(DCH, nblks[3])), nblks[3]),
            ]
            for b, t0, t1 in plan:
                if t0 >= t1:
                    continue
                nc.sync.dma_start(
                    kt_sb[b][:, t0 * BLK : t1 * BLK],
                    kt[b][:, t0 * BLK : t1 * BLK],
                )
                nc.sync.dma_start(
                    v_sb[b][:, t0 * DV : t1 * DV],
                    v[b][:, t0 * DV : t1 * DV],
                )

            # process chunks in data-arrival order
            chunk_list = []
            for b, t0, t1 in plan:
                cs = _chunks(nblks[b])
                for ci, (j0, j1) in enumerate(cs):
                    if t0 <= j0 < t1:
                        chunk_list.append(
                            (b, j0, j1, ci == 0, ci == len(cs) - 1)
                        )

            def scores(ch):
                b, j0, j1, first, last = ch
                w = (j1 - j0) * QR
                psT = psTpool.tile(
                    [BLK, CH * QR], F32, name=f"psT{b}_{j0}", tag="psT"
                )
                for j in range(j0, j1):
                    nc.tensor.matmul(
                        psT[:, (j - j0) * QR : (j - j0 + 1) * QR],
                        lhsT=kt_sb[b][:, j * BLK : (j + 1) * BLK],
                        rhs=qt_t[:, b * QR : (b + 1) * QR],
                        start=True,
                        stop=True,
                    )
                nc.scalar.activation(
                    p_u[b][:, j0 * QR : j1 * QR],
                    psT[:, :w],
                    mybir.ActivationFunctionType.Exp,
                )
                # zero the masked tail (lives in the last two blocks)
                for i2 in range(2):
                    kb = nblks[b] - 2 + i2
                    if j0 <= kb < j1:
                        sl = slice(kb * QR, (kb + 1) * QR)
                        nc.vector.tensor_mul(
                            p_u[b][:, sl],
                            p_u[b][:, sl],
                            mask_t[:, (b * 2 + i2) * QR : (b * 2 + i2 + 1) * QR],
                        )

            def _den(ch):
                # denominator partials: [1, 256] += ones^T @ p (j-slot =
                # j mod 16, the host sums the 16 slots).  Must never carry
                # start=True: the batch's first PV matmul performs the only
                # bank-clear; with the has_written bits cleared the first
                # den matmul overwrites (bit unset) and later ones
                # accumulate (bit set) — per-element semantics.
                b, j0, j1, first, last = ch
                DW = DEN_J * QR
                for p0 in range(j0 * QR, j1 * QR, DW):
                    p1 = min(p0 + DW, j1 * QR)
                    nc.tensor.matmul(
                        ps[b][0:1, QR : QR + (p1 - p0)],
                        lhsT=ones_t[:],
                        rhs=p_u[b][:, p0:p1],
                        start=False,
                        stop=(last and p1 == j1 * QR),
                        skip_group_check=True,
                    )

            def pv(ch):
                b, j0, j1, first, last = ch
                w = (j1 - j0) * QR
                if last and not first:
                    # run the den matmul before the PV matmuls in the final
                    # chunk so the den copy + write overlap the PV burst on
                    # the terminal chain (den has the same p_u dependency)
                    _den(ch)
                for j in range(j0, j1):
                    nc.tensor.matmul(
                        ps[b][:, 0:QR],
                        lhsT=v_sb[b][:, j * DV : (j + 1) * DV],
                        rhs=p_u[b][:, j * QR : (j + 1) * QR],
                        start=(first and j == j0),
                        stop=(last and j == j1 - 1),
                        skip_group_check=True,
                    )
                if not (last and not first):
                    _den(ch)
                if last:
                    # PSUM->SBUF copies on VECTOR (keeps the tail ACTs on
                    # scalar unblocked); the writes go via SYNC, whose K/V
                    # issue queue is drained by the time epilogues run.
                    den_sb = spool.tile([1, DEN_J * QR], F32, name=f"dsb{b}")
                    nc.vector.tensor_scalar_add(
                        den_sb[:], ps[b][0:1, QR : QR + DEN_J * QR], 0.0
                    )
                    nc.sync.dma_start(den[b], den_sb[:])
                    outT_sb = spool.tile([DV, QR], F32, name=f"osb{b}")
                    nc.vector.tensor_scalar_add(outT_sb[:], ps[b][:, 0:QR], 0.0)
                    nc.sync.dma_start(outT[b], outT_sb[:])

            # Software pipeline on the PE: scores(i+1) is issued before
            # pv(i) so the PE always has score matmuls to chew on while the
            # scalar engine's exp for chunk i completes.  Exception: when
            # chunk i closes a batch, issue pv(i) immediately — the next
            # batch's scores wait on not-yet-streamed K and would stall the
            # ready PV (and the batch epilogue) behind them in the in-order
            # PE queue.
            prev = None
            for ch in chunk_list:
                if prev is not None and prev[4]:
                    pv(prev)
                    prev = None
                scores(ch)
                if prev is not None:
                    pv(prev)
                prev = ch
            pv(prev)

    nc.compile()
    return nc


def _shard_inputs(Q, K, V, cache_seqlens, nblks):
    """Per-core input maps. Core c owns KV head c (query heads 4c..4c+3)."""
    scale = 1.0 / np.sqrt(D)
    qs = (np.asarray(Q, dtype=np.float32) * scale).astype(MM_NP)
    K = np.asarray(K, dtype=np.float32)
    V = np.asarray(V, dtype=np.float32)
    cs = np.asarray(cache_seqlens).astype(np.int64)

    ones = np.ones((BLK, 1), MM_NP)

    # 0/1 mask for the last two blocks of each batch: [128, (b, i, q)]
    mask = np.zeros((BLK, B, 2, QR), np.float32)
    sl = np.arange(BLK)
    m_of_r = np.arange(QR) // G
    for b in range(B):
        for i in range(2):
            s = (nblks[b] - 2 + i) * BLK + sl  # absolute kv position
            valid = s[:, None] <= (cs[b] - SQ + m_of_r)[None, :]
            mask[:, b, i, :] = valid.astype(np.float32)
    mask = np.ascontiguousarray(mask.reshape(BLK, B * 2 * QR)).astype(MM_NP)

    in_maps = []
    for c in range(NCORES):
        m = {
            "qt": np.ascontiguousarray(
                qs[:, :, c * G : (c + 1) * G, :].transpose(3, 0, 1, 2)
            ).reshape(D, B * QR),
            "mask": mask,
            "ones": ones,
        }
        for b in range(B):
            nb = nblks[b]
            sb = nb * BLK
            m[f"kt{b}"] = np.ascontiguousarray(K[b, :sb, c, :].T).astype(MM_NP)
            # swizzle V to the SBUF block image: [sl, (kb, dv)]
            m[f"v{b}"] = np.ascontiguousarray(
                V[b, :sb, c, :].reshape(nb, BLK, DV).transpose(1, 0, 2)
            ).reshape(BLK, nb * DV).astype(MM_NP)
        in_maps.append(m)
    return in_maps


def _run(Q, K, V, cache_seqlens, trace=False, trace_cores=None):
    cs = np.asarray(cache_seqlens).astype(np.int64)
    nblks = tuple(
        int(min((int(cs[b]) + BLK - 1) // BLK, SMAX // BLK)) for b in range(B)
    )
    nc = _build(nblks)
    in_maps = _shard_inputs(Q, K, V, cache_seqlens, nblks)
    res = bass_utils.run_bass_kernel_spmd(
        nc,
        in_maps,
        core_ids=list(range(NCORES)),
        trace=trace,
        trace_cores=trace_cores,
    )
    out = np.empty((B, SQ, H, DV), np.float32)
    for c in range(NCORES):
        r = res.results[c]
        for b in range(B):
            num = r["outT"][b].astype(np.float32)  # [DV, QR]
            d = r["den"][b].reshape(DEN_J, QR).astype(np.float32).sum(0)  # [QR]
            o = (num / d[None, :]).T  # [QR, DV]
            out[b, :, c * G : (c + 1) * G, :] = o.reshape(SQ, G, DV)
    return out, res


def kernel(Q, K, V, cache_seqlens):
    out, _ = _run(Q, K, V, cache_seqlens)
    return out


# revision 40
# speedup vs baseline: 1.1645x; 1.1645x over previous
"""Trainium2 Bass kernel: GQA attention with KV cache (decode, Sq=4).

Problem shapes (hardcoded):
  Q [4, 4, 32, 128] f32, K [4, 8192, 8, 128] f32, V [4, 8192, 8, 128] f32,
  cache_seqlens [4] i32 in [4096, 8192].  Output [4, 4, 32, 128] f32.

Sharding: tensor-parallel over the 8 KV heads — core c owns KV head c and
its 4 grouped query heads, for all 4 batches.  Every core therefore does
identical work regardless of cache_seqlens skew.

Per (batch) unit, per 16-block chunk of the KV cache (block = 128 positions):
  scoresT[s,q] = (K_blk^T as lhsT stationary) x (Q^T moving [128,16])
  p = exp(scoresT)            (no max-subtraction needed: scores ~ N(0,1))
  denom partials: ones[128,1] lhsT x p chunk -> [1, 256] PSUM accumulation
  outT[dv,q] += (V_blk [128,128] as lhsT stationary) x (p_blk moving [128,16])
With V stationary the PV matmul streams only 16 columns per block (like the
scores matmul), so the PE runs at ~34ns/block for both phases instead of
128-column streaming.  The kernel returns the transposed numerator and the
denominator partials; the final division + transpose happen on the host.

All K/V transfers are issued up front on the sync HWDGE ring into fully-
resident SBUF tiles (~115KB/partition total), so the stream runs flat at
the ~420 GB/s per-core HBM ceiling with no buffer-recycling stalls.
Masked tail (last <=2 blocks) is zeroed on p with a host-built 0/1 mask.
Blocks past ceil(cache_seqlens/128)*128 are skipped entirely (sparse win).
"""

import functools

import numpy as np
import ml_dtypes

import concourse.bacc as bacc
import concourse.mybir as mybir
import concourse.tile as tile
from concourse import bass_utils

B, SQ, H, HKV, D, DV, SMAX = 4, 4, 32, 8, 128, 128, 8192
G = H // HKV  # 4 query heads per KV head
QR = SQ * G  # 16 query rows per (batch, kv-head) unit
BLK = 128  # kv positions per matmul block
CH = 16  # blocks per chunk (PSUM-score/exp granularity)
DEN_J = 16  # denominator j-slot count (j mod DEN_J); region [1, DEN_J*QR] in PSUM
NCORES = 8

MM_DT = mybir.dt.bfloat16
MM_NP = np.dtype(ml_dtypes.bfloat16)
F32 = mybir.dt.float32


def _chunks(nblk):
    return [(j0, min(j0 + CH, nblk)) for j0 in range(0, nblk, CH)]


def _lean_drain_and_barrier(self, tick_clock, wait_clock):
    """Cheaper TileContext exit: drain + one barrier + sem/DMA reset, without
    the trailing all-engine barrier.  Nothing follows the TileContext in this
    program, and nrt waits for every engine to halt before re-execution, so
    the semaphore clears still happen-before any subsequent run."""
    from concourse.vector_clock import ScopedClock

    drain_inst = self.nc.sync.drain()
    wait_clock.add_sem_waits(
        drain_inst.ins, ScopedClock({None: tick_clock.global_clock})
    )
    self.nc.all_engine_barrier()
    popped = self.nc._tile_sem_poison_stack.pop()
    assert popped is self._sem_poison
    self.nc.clear_and_free_semaphores(list(self.sems.allocated().values()))


@functools.lru_cache(maxsize=4)
def _build(nblks: tuple[int, ...]):
    """Build + compile the per-core SPMD program for given per-batch block counts."""
    nc = bacc.Bacc("TRN2", target_bir_lowering=False, debug=False)

    qt = nc.dram_tensor("qt", [D, B * QR], MM_DT, kind="ExternalInput")
    kt = [
        nc.dram_tensor(f"kt{b}", [D, n * BLK], MM_DT, kind="ExternalInput")
        for b, n in enumerate(nblks)
    ]
    # V arrives host-swizzled to the SBUF image: [sl, kb*DV] with
    # v[sl, kb*DV + dv] = V[128*kb + sl, dv] — flat contiguous runs/partition.
    v = [
        nc.dram_tensor(f"v{b}", [BLK, n * DV], MM_DT, kind="ExternalInput")
        for b, n in enumerate(nblks)
    ]
    mask = nc.dram_tensor("mask", [BLK, B * 2 * QR], MM_DT, kind="ExternalInput")
    ones = nc.dram_tensor("ones", [BLK, 1], MM_DT, kind="ExternalInput")
    outT = nc.dram_tensor("outT", [B, DV, QR], F32, kind="ExternalOutput")
    den = nc.dram_tensor("den", [B, 1, DEN_J * QR], F32, kind="ExternalOutput")

    tile.TileContext._drain_and_barrier = _lean_drain_and_barrier
    with tile.TileContext(nc) as tc:
        with (
            tc.tile_pool(name="const", bufs=1) as cpool,
            tc.tile_pool(name="ktp", bufs=1) as kpool,
            tc.tile_pool(name="vp", bufs=1) as vpool,
            tc.tile_pool(name="pp", bufs=1) as ppool,
            tc.tile_pool(name="small", bufs=1) as spool,
            tc.tile_pool(name="psT", bufs=4, space="PSUM") as psTpool,
            tc.tile_pool(name="psO", bufs=1, space="PSUM") as psOpool,
        ):
            # qt rides first on the sync ring (needed by the first scores);
            # mask/ones go via gpsimd SWDGE so they never delay the K/V
            # stream issues.
            qt_t = cpool.tile([D, B * QR], MM_DT, tag="qt")
            nc.sync.dma_start(qt_t[:], qt[:])
            mask_t = cpool.tile([BLK, B * 2 * QR], MM_DT, tag="mask")
            nc.gpsimd.dma_start(mask_t[:], mask[:])
            ones_t = cpool.tile([BLK, 1], MM_DT, tag="ones")
            nc.gpsimd.dma_start(ones_t[:], ones[:])

            kt_sb, v_sb, p_u, ps = [], [], [], []
            for b in range(B):
                n = nblks[b]
                kt_sb.append(
                    kpool.tile([D, n * BLK], MM_DT, name=f"ktsb{b}")
                )
                v_sb.append(vpool.tile([BLK, n * DV], MM_DT, name=f"vsb{b}"))
                p_u.append(ppool.tile([BLK, n * QR], MM_DT, name=f"pu{b}"))
                # [:, :QR] holds the outT accumulator; [0:1, QR:QR+DEN_J*QR]
                # holds the denominator partial sums (per j-slot x q).
                ps.append(
                    psOpool.tile([BLK, QR + DEN_J * QR], F32, name=f"ps{b}")
                )

            # All K/V transfers up front on the SYNC ring only, interleaved
            # K-then-V per wave.  Sync does nothing else, so the per-engine
            # DMA-semaphore pacing (first ~8 issues fire back-to-back, later
            # ones block the engine until an older transfer completes) is
            # harmless there.  Putting transfers on the scalar ring instead
            # stalls the ACTIVATEs queued behind the blocked issues —
            # measured as a 7-9us regression, twice.  ~1MB transfers,
            # round-robined across batches per wave so the four per-batch
            # tails land staggered at the stream end and their
            # scores->exp->PV chains overlap.
            DCH = 2 * CH  # blocks per DMA transfer (1MB)
            ntr = max((nblks[b] + DCH - 1) // DCH for b in range(B))

            # b0/b1 go as one monolithic K/V transfer each — their chunk
            # chains hide under the stream, and halving the early issue
            # count (each DMA_DIRECT2D costs 0.6-2us of sync-engine time)
            # gets the ring to full depth sooner, flattening the ramp dip.
            # b2/b3 keep two waves so the terminal chain gates on a small
            # final piece.
            plan = [
                (0, 0, nblks[0]),
                (1, 0, nblks[1]),
                (2, 0, min(DCH, nblks[2])),
                (3, 0, min(DCH, nblks[3])),
                (2, min(DCH, nblks[2]), nblks[2]),
                (3, min(DCH, nblks[3]), nblks[3]),
            ]
            for b, t0, t1 in plan:
                if t0 >= t1:
                    continue
                nc.sync.dma_start(
                    kt_sb[b][:, t0 * BLK : t1 * BLK],
                    kt[b][:, t0 * BLK : t1 * BLK],
                )
                nc.sync.dma_start(
                    v_sb[b][:, t0 * DV : t1 * DV],
                    v[b][:, t0 * DV : t1 * DV],
                )

            # process chunks in data-arrival order
            chunk_list = []
            for b, t0, t1 in plan:
                cs = _chunks(nblks[b])
                for ci, (j0, j1) in enumerate(cs):
                    if t0 <= j0 < t1:
                        chunk_list.append(
                            (b, j0, j1, ci == 0, ci == len(cs) - 1)
                        )

            def scores(ch):
                b, j0, j1, first, last = ch
                w = (j1 - j0) * QR
                psT = psTpool.tile(
                    [BLK, CH * QR], F32, name=f"psT{b}_{j0}", tag="psT"
                )
                for j in range(j0, j1):
                    nc.tensor.matmul(
                        psT[:, (j - j0) * QR : (j - j0 + 1) * QR],
                        lhsT=kt_sb[b][:, j * BLK : (j + 1) * BLK],
                        rhs=qt_t[:, b * QR : (b + 1) * QR],
                        start=True,
                        stop=True,
                    )
                nc.scalar.activation(
                    p_u[b][:, j0 * QR : j1 * QR],
                    psT[:, :w],
                    mybir.ActivationFunctionType.Exp,
                )
                # zero the masked tail (lives in the last two blocks)
                for i2 in range(2):
                    kb = nblks[b] - 2 + i2
                    if j0 <= kb < j1:
                        sl = slice(kb * QR, (kb + 1) * QR)
                        nc.vector.tensor_mul(
                            p_u[b][:, sl],
                            p_u[b][:, sl],
                            mask_t[:, (b * 2 + i2) * QR : (b * 2 + i2 + 1) * QR],
                        )

            def _den(ch):
                # denominator partials: [1, 256] += ones^T @ p (j-slot =
                # j mod 16, the host sums the 16 slots).  Must never carry
                # start=True: the batch's first PV matmul performs the only
                # bank-clear; with the has_written bits cleared the first
                # den matmul overwrites (bit unset) and later ones
                # accumulate (bit set) — per-element semantics.
                b, j0, j1, first, last = ch
                DW = DEN_J * QR
                for p0 in range(j0 * QR, j1 * QR, DW):
                    p1 = min(p0 + DW, j1 * QR)
                    nc.tensor.matmul(
                        ps[b][0:1, QR : QR + (p1 - p0)],
                        lhsT=ones_t[:],
                        rhs=p_u[b][:, p0:p1],
                        start=False,
                        stop=(last and p1 == j1 * QR),
                        skip_group_check=True,
                    )

            def pv(ch):
                b, j0, j1, first, last = ch
                w = (j1 - j0) * QR
                if last and not first:
                    # run the den matmul before the PV matmuls in the final
                    # chunk so the den copy + write overlap the PV burst on
                    # the terminal chain (den has the same p_u dependency)
                    _den(ch)
                for j in range(j0, j1):
                    nc.tensor.matmul(
                        ps[b][:, 0:QR],
                        lhsT=v_sb[b][:, j * DV : (j + 1) * DV],
                        rhs=p_u[b][:, j * QR : (j + 1) * QR],
                        start=(first and j == j0),
                        stop=(last and j == j1 - 1),
                        skip_group_check=True,
                    )
                if not (last and not first):
                    _den(ch)
                if last:
                    # PSUM->SBUF copies on VECTOR (keeps the tail ACTs on
                    # scalar unblocked); the writes go via SYNC, whose K/V
                    # issue queue is drained by the time epilogues run.
                    den_sb = spool.tile([1, DEN_J * QR], F32, name=f"dsb{b}")
                    nc.vector.tensor_scalar_add(
                        den_sb[:], ps[b][0:1, QR : QR + DEN_J * QR], 0.0
                    )
                    nc.sync.dma_start(den[b], den_sb[:])
                    outT_sb = spool.tile([DV, QR], F32, name=f"osb{b}")
                    nc.vector.tensor_scalar_add(outT_sb[:], ps[b][:, 0:QR], 0.0)
                    nc.sync.dma_start(outT[b], outT_sb[:])

            # Software pipeline on the PE: scores(i+1) is issued before
            # pv(i) so the PE always has score matmuls to chew on while the
            # scalar engine's exp for chunk i completes.  Exception: when
            # chunk i closes a batch, issue pv(i) immediately — the next
            # batch's scores wait on not-yet-streamed K and would stall the
            # ready PV (and the batch epilogue) behind them in the in-order
            # PE queue.
            prev = None
            for ch in chunk_list:
                if prev is not None and prev[4]:
                    pv(prev)
                    prev = None
                scores(ch)
                if prev is not None:
                    pv(prev)
                prev = ch
            pv(prev)

    nc.compile()
    return nc


def _shard_inputs(Q, K, V, cache_seqlens, nblks):
    """Per-core input maps. Core c owns KV head c (query heads 4c..4c+3)."""
    scale = 1.0 / np.sqrt(D)
    qs = (np.asarray(Q, dtype=np.float32) * scale).astype(MM_NP)
    K = np.asarray(K, dtype=np.float32)
    V = np.asarray(V, dtype=np.float32)
    cs = np.asarray(cache_seqlens).astype(np.int64)

    ones = np.ones((BLK, 1), MM_NP)

    # 0/1 mask for the last two blocks of each batch: [128, (b, i, q)]
    mask = np.zeros((BLK, B, 2, QR), np.float32)
    sl = np.arange(BLK)
    m_of_r = np.arange(QR) // G
    for b in range(B):
        for i in range(2):
            s = (nblks[b] - 2 + i) * BLK + sl  # absolute kv position
            valid = s[:, None] <= (cs[b] - SQ + m_of_r)[None, :]
            mask[:, b, i, :] = valid.astype(np.float32)
    mask = np.ascontiguousarray(mask.reshape(BLK, B * 2 * QR)).astype(MM_NP)

    in_maps = []
    for c in range(NCORES):
        m = {
            "qt": np.ascontiguousarray(
                qs[:, :, c * G : (c + 1) * G, :].transpose(3, 0, 1, 2)
            ).reshape(D, B * QR),
            "mask": mask,
            "ones": ones,
        }
        for b in range(B):
            nb = nblks[b]
            sb = nb * BLK
            m[f"kt{b}"] = np.ascontiguousarray(K[b, :sb, c, :].T).astype(MM_NP)
            # swizzle V to the SBUF block image: [sl, (kb, dv)]
            m[f"v{b}"] = np.ascontiguousarray(
                V[b, :sb, c, :].reshape(nb, BLK, DV).transpose(1, 0, 2)
            ).reshape(BLK, nb * DV).astype(MM_NP)
        in_maps.append(m)
    return in_maps


def _run(Q, K, V, cache_seqlens, trace=False, trace_cores=None):
    cs = np.asarray(cache_seqlens).astype(np.int64)
    nblks = tuple(
        int(min((int(cs[b]) + BLK - 1) // BLK, SMAX // BLK)) for b in range(B)
    )
    nc = _build(nblks)
    in_maps = _shard_inputs(Q, K, V, cache_seqlens, nblks)
    res = bass_utils.run_bass_kernel_spmd(
        nc,
        in_maps,
        core_ids=list(range(NCORES)),
        trace=trace,
        trace_cores=trace_cores,
    )
    out = np.empty((B, SQ, H, DV), np.float32)
    for c in range(NCORES):
        r = res.results[c]
        for b in range(B):
            num = r["outT"][b].astype(np.float32)  # [DV, QR]
            d = r["den"][b].reshape(DEN_J, QR).astype(np.float32).sum(0)  # [QR]
            o = (num / d[None, :]).T  # [QR, DV]
            out[b, :, c * G : (c + 1) * G, :] = o.reshape(SQ, G, DV)
    return out, res


def kernel(Q, K, V, cache_seqlens):
    out, _ = _run(Q, K, V, cache_seqlens)
    return out
